# revision 30
# baseline (speedup 1.0000x reference)
"""MHSA3D Trainium2 kernel: 8-way head-parallel flash-style attention.

Problem (hardcoded): B=1, C=128, D=H=W=16 -> N=4096 tokens, 8 heads,
dh=16, dv=128.  Each of the 8 NeuronCores computes one head end-to-end:
qkv projection (its head's slice), S^T = k''^T q'' logits in [j, i]
layout, exp on ScalarE (no max subtraction -- fp32 exp cannot overflow
for this data), PV accumulation with a prepended ones-column producing
the softmax denominator, then normalize.

The ScalarE exp stream is the roofline (N^2/128 lanes / 1.2 GHz =
109 us/core); everything is organized to keep that stream dense:
- exp tiles alternate [128,2048] / [128,1536] (4+3 PSUM banks,
  single-buffered each => hardware double-buffering across the pair),
  amortizing the ~293 ns per-ACTIVATE fixed cost over 9 instead of 16
  instructions per 512-column eighth.
- The 8th PSUM bank holds the qkv projection staging (two alternating
  48-row regions), then the PV accumulator (rows 0-47) plus the
  recip-broadcast scratch (rows 64-79).
- One K=128 M=48 projection matmul per 512-col chunk; ONE DVE add
  extracts q''|k''|v into a [48, N] fp16 plane (biases host-folded into
  a [48, N] fp16 plane: q-bias bcast / k-bias+positional emb / zeros).
- The qk contraction runs at K=48 over that plane directly (q rows
  0-15 pair with k'' rows 0-15 of the stationary tile; rows 16-47 are
  zero there, masking the k/v rows).  K=48 is the smallest contraction
  that keeps the PE HAM activity monitor at 2.4 GHz.
- k'' is peeled into the stationary tile by 4 deadline-batched
  SBUF->SBUF DMAs; v^T is built by two [16,2048] xbar-transpose DMAs
  (v rows -> [128, 16, 16] blocks, 32B-aligned destinations).
- HBM loads are split across the sync and scalar HWDGE queues (one
  queue sustains only ~60 GB/s; gpsimd DMA is a far slower software
  DGE) in chain-consumption order.
- Softmax reciprocal via the custom-DVE reciprocal_approx_fast (~5x
  faster than DVE reciprocal, 51 ULP); the denominator lands on
  partition 0 (ones-column first in the PV stationary layout) because
  engine APs off partition 0 must be 32-aligned and <= 32 rows.
- PV for group g is emitted 3 groups late so the in-order PE FIFO
  never head-of-line blocks on the exp; per-eighth normalize tails are
  deferred into the next eighth (copy+recip at gi==2, broadcast-matmul
  multiply, bv add and output DMA at gi==6).

Host side: fold the 1/sqrt(dh) scale into wq/bq, fold b_k into the
positional-embedding plane, slice per-head weights, run the SPMD
program on cores 0-7, and concatenate the per-head [16, N] outputs.
"""

import numpy as np

NHEADS = 8
DV = 128
DH = DV // NHEADS  # 16
C = 128
N = 4096
ECOLS = 512        # i-columns handled per output tile ("eighth")
NE = N // ECOLS    # 8
JW = 128           # keys per j-block
NJB = N // JW      # 32
VS = 48            # vaugT per-block stride (ones | 31 zeros | 16 v)
LAG = 3            # PV groups trail the exp stream by this many groups

# j-block grouping per eighth: alternating 4-block (2048-col) and
# 3-block (1536-col) exp tiles; 4+3 PSUM banks double-buffer.
GSIZES = [4, 3, 4, 3, 4, 3, 4, 3, 4]
GROUPS = []
_j = 0
for _s in GSIZES:
    GROUPS.append(tuple(range(_j, _j + _s)))
    _j += _s
assert _j == NJB

_compiled = None


def _build_program():
    import concourse.bacc as bacc
    import concourse.mybir as mybir
    import concourse.tile as tile

    f32 = mybir.dt.float32
    bf16 = mybir.dt.bfloat16
    fp16 = mybir.dt.float16
    EXP = mybir.ActivationFunctionType.Exp
    ADD = mybir.AluOpType.add
    MULT = mybir.AluOpType.mult

    nc = bacc.Bacc("TRN2", target_bir_lowering=False, debug=False,
                   num_devices=NHEADS)

    x_d = nc.dram_tensor("x", [C, N], fp16, kind="ExternalInput")
    # w cols: 0-15 wq*scale, 16-31 wk, 32-47 wv
    w_d = nc.dram_tensor("w", [C, 48], fp16, kind="ExternalInput")
    # bias plane rows: 0-15 bq*scale (bcast), 16-31 bk+emb, 32-47 zero
    b_d = nc.dram_tensor("bias", [48, N], fp16, kind="ExternalInput")
    bv_d = nc.dram_tensor("bv", [DH, 1], f32, kind="ExternalInput")
    o_d = nc.dram_tensor("out", [DH, N], f32, kind="ExternalOutput")

    with tile.TileContext(nc) as tc:
        with (
            tc.tile_pool(name="const", bufs=1) as const,
            tc.tile_pool(name="pt", bufs=3) as ptp,
            tc.tile_pool(name="o", bufs=3) as op,
            tc.tile_pool(name="stA", bufs=1, space="PSUM") as stAp,
            tc.tile_pool(name="stB", bufs=1, space="PSUM") as stBp,
            tc.tile_pool(name="acc", bufs=1, space="PSUM") as accp,
        ):
            x_s = const.tile([C, N], fp16)
            w_s = const.tile([C, 48], fp16)
            bv_s = const.tile([DH, 1], f32)
            biasf = const.tile([48, N], fp16)
            # qz rows: 0-15 q''; 16-31 k''; 32-47 v (all three from one
            # DVE add -- rows 16-47 are masked by the kz zero rows in the
            # K=96 contraction, and v doubles as the transpose source);
            # 48-95 zero.  kz rows: 0-15 k''; 16-95 exact zero.
            qz_all = const.tile([48, N], fp16)
            kz_all = const.tile([48, N], fp16)
            vaugT = const.tile([128, VS * NJB], fp16)
            ones16 = const.tile([1, DH], f32)
            zerob = const.tile([128, 1], f32)
            scratch1 = const.tile([128, 1], f32)
            # single PSUM bank: qkv projection staging, then the PV
            # accumulator rows 0-16 + recip-broadcast rows 64-79.
            acc_full = accp.tile([128, 512], f32)

            # --- startup: memsets, DMAs, exp-table warm ---
            nc.gpsimd.memset(zerob[:], 0.0)
            nc.gpsimd.memset(ones16[:], 1.0)
            # Warm the exp table set while DMAs run.
            nc.scalar.activation(scratch1[:], zerob[:], EXP, bias=zerob[:])
            # K=48 contraction (the smallest that keeps the PE HAM at
            # 2.4 GHz): qz rows 0-47 are live q|k|v data, kz rows 16-47
            # mask the k/v rows.  Engine APs off partition 0 need
            # 32-aligned starts, so rows 32-47 get a memset and rows
            # 16-31 a DMA copy from them.
            nc.vector.memset(kz_all[2 * DH:3 * DH, :], 0.0)
            nc.gpsimd.dma_start(kz_all[DH:2 * DH, :], kz_all[2 * DH:3 * DH, :])

            # Per-block layout: col 0 = ones (denominator -> acc row 0,
            # partition-0-aligned for the DVE reciprocal), cols 1-31
            # zero, cols 32-47 = v^T (32B-aligned xbar-transpose dsts).
            va3 = vaugT[:].rearrange("p (c s) -> p c s", s=VS)
            nc.vector.memset(va3[:, :, 0:1], 1.0)
            nc.vector.memset(va3[:, :, 1:2 * DH], 0.0)

            # HBM loads split across the two HWDGE queues (gpsimd DMA is
            # a slow software DGE, and one queue sustains only ~60 GB/s,
            # transfer-serialized): sync takes the chunks that gate the
            # first half of the projection chain, scalar (idle until the
            # exp stream starts) takes the rest.
            nc.sync.dma_start(x_s[:, 0:1024], x_d.ap()[:, 0:1024])
            nc.sync.dma_start(w_s[:], w_d.ap())
            nc.sync.dma_start(biasf[:, 0:2048], b_d.ap()[:, 0:2048])
            nc.sync.dma_start(x_s[:, 1024:2048], x_d.ap()[:, 1024:2048])
            nc.scalar.dma_start(x_s[:, 2048:3072], x_d.ap()[:, 2048:3072])
            nc.scalar.dma_start(biasf[:, 2048:4096], b_d.ap()[:, 2048:4096])
            nc.scalar.dma_start(x_s[:, 3072:4096], x_d.ap()[:, 3072:4096])
            nc.gpsimd.dma_start(bv_s[:], bv_d.ap())

            # --- qkv projection: one K=128 M=48 matmul per 512-col
            # chunk, one DVE add folding all three biases, one SBUF->SBUF
            # DMA peeling k'' into the zero-padded stationary tile, one
            # xbar-transpose DMA building this chunk's v^T blocks.
            # Chunks 0-1 stage in acc_full[0:48] (freed early for the PV
            # accumulator); chunks 2-7 serialize through acc_full[64:112].
            def emit_proj(c):
                cs = slice(c * 512, (c + 1) * 512)
                if c % 2 == 0:
                    ps = acc_full[0:48, :]
                    tp = None
                else:
                    ps = acc_full[64:112, :]
                    tp = (0, 64)
                nc.tensor.matmul(ps, lhsT=w_s[:, 0:48], rhs=x_s[:, cs],
                                 start=True, stop=True, tile_position=tp)
                nc.vector.tensor_tensor(qz_all[0:3 * DH, cs], ps[:],
                                        biasf[:, cs], ADD)
                # k'' peel-off on the sync HWDGE queue, batched by
                # deadline (early chunks gate the exp stream start, late
                # ones only their own qk group); v^T transposes batched
                # per half ([16, 2048] -> [128, 16, 16]).
                if c in (0, 1, 3, 7):
                    lo = {0: 0, 1: 512, 3: 1024, 7: 2048}[c]
                    ks = slice(lo, (c + 1) * 512)
                    nc.sync.dma_start(kz_all[0:DH, ks],
                                      qz_all[DH:2 * DH, ks])
                if c in (3, 7):
                    ts2 = slice((c - 3) * 512, (c + 1) * 512)
                    nc.sync.dma_start_transpose(
                        va3[:, 4 * (c - 3):4 * (c + 1), 2 * DH:3 * DH],
                        qz_all[2 * DH:3 * DH, ts2])

            def make_pv(pt, jbs, acc, start, stop):
                def emit():
                    for t, jb in enumerate(jbs):
                        nc.tensor.matmul(
                            acc,
                            lhsT=vaugT[:, VS * jb:VS * (jb + 1)],
                            rhs=pt[:, 512 * t:512 * (t + 1)],
                            start=(start and t == 0),
                            stop=(stop and t == len(jbs) - 1),
                            skip_group_check=True)
                return emit

            def make_tail_a(acc):
                o17 = op.tile([3 * DH, ECOLS], f32, tag="o17")
                r = op.tile([1, ECOLS], f32, tag="r")

                def emit():
                    nc.vector.tensor_copy(o17[:], acc)
                    nc.vector.reciprocal_approx_fast(r[:], o17[0:1, :])
                return emit, o17, r

            def make_tail_b(o17, r, es):
                def emit():
                    # broadcast r across 16 partitions via a ones-matmul
                    # into spare partitions of the accumulator bank.
                    bc = acc_full[64:64 + DH, :]
                    nc.tensor.matmul(bc, lhsT=ones16[:], rhs=r[:],
                                     start=True, stop=True,
                                     tile_position=(0, 64),
                                     skip_group_check=True)
                    ost = op.tile([DH, ECOLS], f32, tag="ost")
                    nc.vector.tensor_tensor(ost[:], o17[2 * DH:3 * DH, :], bc, MULT)
                    nc.vector.tensor_scalar_add(ost[:], ost[:], bv_s[:])
                    if es.start == (NE - 1) * ECOLS:
                        # The scalar queue is idle after the last exp;
                        # sync may still owe ring credits to the earlier
                        # output DMAs.
                        nc.scalar.dma_start(o_d.ap()[:, es], ost[:])
                    else:
                        nc.sync.dma_start(o_d.ap()[:, es], ost[:])
                return emit

            from collections import deque
            pend = deque()
            pending_a = None
            pending_b = None
            acc48 = acc_full[0:3 * DH, :]

            state = {"pending_a": None, "pending_b": None}

            def emit_group(e, gi, jbs, lag):
                njb = len(jbs)
                fw = 512 * njb
                if njb == 4:
                    st = stAp.tile([128, 2048], f32, tag="A")
                else:
                    st = stBp.tile([128, 1536], f32, tag="B")
                for t, jb in enumerate(jbs):
                    nc.tensor.matmul(
                        st[:, 512 * t:512 * (t + 1)],
                        lhsT=kz_all[:, jb * JW:(jb + 1) * JW],
                        rhs=qz_all[:, e * ECOLS:(e + 1) * ECOLS],
                        start=True, stop=True)
                pt = ptp.tile([128, fw], bf16, tag=("ptA" if njb == 4
                                                    else "ptB"))
                nc.scalar.activation(pt[:], st[:], EXP, bias=zerob[:])
                while len(pend) >= lag:
                    pend.popleft()()
                # The prior eighth's accumulator must be copied out
                # (tail_a) before this eighth's start=True PV clears it.
                # With lag==1 that clear is emitted at gi==1, so fire
                # tail_a at gi==0; with lag==3 it is emitted at gi==3 and
                # gi==2 keeps the copy off the PE's heels.
                tail_a_gi = 2 if lag >= 3 else 0
                if state["pending_a"] is not None and gi == tail_a_gi:
                    state["pending_a"]()
                    state["pending_a"] = None
                if state["pending_b"] is not None and gi == 6:
                    state["pending_b"]()
                    state["pending_b"] = None
                pend.append(make_pv(pt, jbs, acc48,
                                    start=(gi == 0),
                                    stop=(gi == len(GROUPS) - 1)))

            # Eighth 0 interleaves the tail of the projection chain with
            # the qk groups so the PE FIFO never sits behind the whole
            # chain; the first five chunks go up front (the alternating
            # even/odd staging regions let their extraction adds pipeline
            # at two chunks in flight).
            for c in range(5):
                emit_proj(c)
            for gi, jbs in enumerate(GROUPS):
                if gi < 3:
                    emit_proj(gi + 5)
                emit_group(0, gi, jbs, LAG)
            emit_a, o17, r = make_tail_a(acc48)
            state["pending_a"] = emit_a
            state["pending_b"] = make_tail_b(o17, r, slice(0, ECOLS))

            for e in range(1, NE):
                es = slice(e * ECOLS, (e + 1) * ECOLS)
                lag = LAG if e < NE - 1 else 1
                for gi, jbs in enumerate(GROUPS):
                    emit_group(e, gi, jbs, lag)
                while pend and e == NE - 1:
                    pend.popleft()()
                emit_a, o17, r = make_tail_a(acc48)
                if state["pending_a"] is not None:
                    state["pending_a"]()
                state["pending_a"] = emit_a
                if state["pending_b"] is not None:
                    state["pending_b"]()
                state["pending_b"] = make_tail_b(o17, r, es)
            while pend:
                pend.popleft()()
            state["pending_a"]()
            state["pending_b"]()

    nc.compile()
    return nc


def _get_program():
    global _compiled
    if _compiled is None:
        _compiled = _build_program()
    return _compiled


def _prepare_core_inputs(x, w_qkv, b_qkv, emb_d, emb_h, emb_w):
    x2 = np.ascontiguousarray(
        np.asarray(x, np.float32).reshape(C, N)).astype(np.float16)
    w_qkv = np.asarray(w_qkv, np.float32)
    b_qkv = np.asarray(b_qkv, np.float32)
    scale = DH ** -0.5
    emb = (np.asarray(emb_d, np.float32)
           + np.asarray(emb_h, np.float32)
           + np.asarray(emb_w, np.float32)).reshape(DH, N)
    in_maps = []
    for h in range(NHEADS):
        qc = slice(h * DH, (h + 1) * DH)
        kc = slice(DV + h * DH, DV + (h + 1) * DH)
        vc = slice(2 * DV + h * DH, 2 * DV + (h + 1) * DH)
        w = np.empty((C, 48), np.float32)
        w[:, 0:16] = w_qkv[:, qc] * scale
        w[:, 16:32] = w_qkv[:, kc]
        w[:, 32:48] = w_qkv[:, vc]
        w = w.astype(np.float16)
        bias = np.zeros((48, N), np.float32)
        bias[0:16, :] = (b_qkv[qc] * scale)[:, None]
        bias[16:32, :] = b_qkv[kc][:, None] + emb
        bias = bias.astype(np.float16)
        bv = np.ascontiguousarray(b_qkv[vc][:, None])
        in_maps.append({"x": x2, "w": w, "bias": bias, "bv": bv})
    return in_maps


def kernel(x, w_qkv, b_qkv, emb_d, emb_h, emb_w):
    from concourse.bass_utils import run_bass_kernel_spmd

    nc = _get_program()
    in_maps = _prepare_core_inputs(x, w_qkv, b_qkv, emb_d, emb_h, emb_w)
    res = run_bass_kernel_spmd(nc, in_maps, list(range(NHEADS)))
    out = np.empty((DV, N), np.float32)
    for h in range(NHEADS):
        out[h * DH:(h + 1) * DH, :] = res.results[h]["out"]
    return out.reshape(1, DV, 16, 16, 16)


# revision 32
# speedup vs baseline: 1.1118x; 1.1118x over previous
"""MHSA3D Trainium2 kernel: 8-way head-parallel flash-style attention.

Problem (hardcoded): B=1, C=128, D=H=W=16 -> N=4096 tokens, 8 heads,
dh=16, dv=128.  Each of the 8 NeuronCores computes one head end-to-end:
qkv projection (its head's slice), S^T = k''^T q'' logits in [j, i]
layout, exp on ScalarE (no max subtraction -- fp32 exp cannot overflow
for this data), PV accumulation with a prepended ones-column producing
the softmax denominator, then normalize.

The ScalarE exp stream is the roofline (N^2/128 lanes / 1.2 GHz =
109 us/core); everything is organized to keep that stream dense:
- exp tiles alternate [128,2048] / [128,1536] (4+3 PSUM banks,
  single-buffered each => hardware double-buffering across the pair),
  amortizing the ~293 ns per-ACTIVATE fixed cost over 9 instead of 16
  instructions per 512-column eighth.
- The 8th PSUM bank holds the qkv projection staging (two alternating
  48-row regions), then the PV accumulator (rows 0-47) plus the
  recip-broadcast scratch (rows 64-79).
- One K=128 M=48 projection matmul per 512-col chunk; ONE DVE add
  extracts q''|k''|v into a [48, N] fp16 plane (biases host-folded into
  a [48, N] fp16 plane: q-bias bcast / k-bias+positional emb / zeros).
- The qk contraction runs at K=48 over that plane directly (q rows
  0-15 pair with k'' rows 0-15 of the stationary tile; rows 16-47 are
  zero there, masking the k/v rows).  K=48 is the smallest contraction
  that keeps the PE HAM activity monitor at 2.4 GHz.
- k'' is peeled into the stationary tile by 4 deadline-batched
  SBUF->SBUF DMAs; v^T is built by two [16,2048] xbar-transpose DMAs
  (v rows -> [128, 16, 16] blocks, 32B-aligned destinations).
- HBM loads are split across the sync and scalar HWDGE queues (one
  queue sustains only ~60 GB/s; gpsimd DMA is a far slower software
  DGE) in chain-consumption order.
- Softmax reciprocal via the custom-DVE reciprocal_approx_fast (~5x
  faster than DVE reciprocal, 51 ULP); the denominator lands on
  partition 0 (ones-column first in the PV stationary layout) because
  engine APs off partition 0 must be 32-aligned and <= 32 rows.
- PV for group g is emitted 3 groups late so the in-order PE FIFO
  never head-of-line blocks on the exp; per-eighth normalize tails are
  deferred into the next eighth (copy+recip at gi==2, broadcast-matmul
  multiply, bv add and output DMA at gi==6).

Host side: fold the 1/sqrt(dh) scale into wq/bq, fold b_k into the
positional-embedding plane, slice per-head weights, run the SPMD
program on cores 0-7, and concatenate the per-head [16, N] outputs.
"""

import numpy as np

NHEADS = 8
DV = 128
DH = DV // NHEADS  # 16
C = 128
N = 4096
ECOLS = 512        # i-columns handled per output tile ("eighth")
NE = N // ECOLS    # 8
JW = 128           # keys per j-block
NJB = N // JW      # 32
VS = 48            # vaugT per-block stride (ones | 31 zeros | 16 v)
LAG = 3            # PV groups trail the exp stream by this many groups

# j-block grouping per eighth: alternating 4-block (2048-col) and
# 3-block (1536-col) exp tiles; 4+3 PSUM banks double-buffer.
GSIZES = [4, 3, 4, 3, 4, 3, 4, 3, 4]
GROUPS = []
_j = 0
for _s in GSIZES:
    GROUPS.append(tuple(range(_j, _j + _s)))
    _j += _s
assert _j == NJB

_compiled = None


def _build_program():
    import concourse.bacc as bacc
    import concourse.mybir as mybir
    import concourse.tile as tile

    f32 = mybir.dt.float32
    bf16 = mybir.dt.bfloat16
    fp16 = mybir.dt.float16
    EXP = mybir.ActivationFunctionType.Exp
    ADD = mybir.AluOpType.add
    MULT = mybir.AluOpType.mult

    nc = bacc.Bacc("TRN2", target_bir_lowering=False, debug=False,
                   num_devices=NHEADS)

    x_d = nc.dram_tensor("x", [C, N], fp16, kind="ExternalInput")
    # w cols: 0-15 wq*scale, 16-31 wk, 32-47 wv
    w_d = nc.dram_tensor("w", [C, 48], fp16, kind="ExternalInput")
    # bias plane rows: 0-15 bq*scale (bcast), 16-31 bk+emb, 32-47 zero
    b_d = nc.dram_tensor("bias", [48, N], fp16, kind="ExternalInput")
    bv_d = nc.dram_tensor("bv", [DH, 1], f32, kind="ExternalInput")
    o_d = nc.dram_tensor("out", [DH, N], f32, kind="ExternalOutput")

    with tile.TileContext(nc) as tc:
        with (
            tc.tile_pool(name="const", bufs=1) as const,
            tc.tile_pool(name="pt", bufs=3) as ptp,
            tc.tile_pool(name="o", bufs=3) as op,
            tc.tile_pool(name="stA", bufs=1, space="PSUM") as stAp,
            tc.tile_pool(name="stB", bufs=1, space="PSUM") as stBp,
            tc.tile_pool(name="acc", bufs=1, space="PSUM") as accp,
        ):
            x_s = const.tile([C, N], fp16)
            w_s = const.tile([C, 48], fp16)
            bv_s = const.tile([DH, 1], f32)
            biasf = const.tile([48, N], fp16)
            # qz rows: 0-15 q''; 16-31 k''; 32-47 v (all three from one
            # DVE add -- rows 16-47 are masked by the kz zero rows in the
            # K=96 contraction, and v doubles as the transpose source);
            # 48-95 zero.  kz rows: 0-15 k''; 16-95 exact zero.
            qz_all = const.tile([48, N], fp16)
            kz_all = const.tile([48, N], fp16)
            vaugT = const.tile([128, VS * NJB], fp16)
            ones16 = const.tile([1, DH], f32)
            zerob = const.tile([128, 1], f32)
            scratch1 = const.tile([128, 1], f32)
            # single PSUM bank: qkv projection staging, then the PV
            # accumulator rows 0-16 + recip-broadcast rows 64-79.
            acc_full = accp.tile([128, 512], f32)

            # --- startup: memsets, DMAs, exp-table warm ---
            nc.gpsimd.memset(zerob[:], 0.0)
            nc.gpsimd.memset(ones16[:], 1.0)
            # Warm the exp table set while DMAs run.
            nc.scalar.activation(scratch1[:], zerob[:], EXP, bias=zerob[:])
            # K=48 contraction (the smallest that keeps the PE HAM at
            # 2.4 GHz): qz rows 0-47 are live q|k|v data, kz rows 16-47
            # mask the k/v rows.  Engine APs off partition 0 need
            # 32-aligned starts, so rows 32-47 get a memset and rows
            # 16-31 a DMA copy from them.
            nc.vector.memset(kz_all[2 * DH:3 * DH, :], 0.0)
            nc.gpsimd.dma_start(kz_all[DH:2 * DH, :], kz_all[2 * DH:3 * DH, :])

            # Per-block layout: col 0 = ones (denominator -> acc row 0,
            # partition-0-aligned for the DVE reciprocal), cols 1-31
            # zero, cols 32-47 = v^T (32B-aligned xbar-transpose dsts).
            va3 = vaugT[:].rearrange("p (c s) -> p c s", s=VS)
            nc.vector.memset(va3[:, :, 0:1], 1.0)
            nc.vector.memset(va3[:, :, 1:2 * DH], 0.0)

            # HBM loads split across the two HWDGE queues (gpsimd DMA is
            # a slow software DGE, and one queue sustains only ~60 GB/s,
            # transfer-serialized): sync takes the chunks that gate the
            # first half of the projection chain, scalar (idle until the
            # exp stream starts) takes the rest.
            nc.sync.dma_start(x_s[:, 0:1024], x_d.ap()[:, 0:1024])
            nc.sync.dma_start(w_s[:], w_d.ap())
            nc.sync.dma_start(biasf[:, 0:2048], b_d.ap()[:, 0:2048])
            nc.sync.dma_start(x_s[:, 1024:2048], x_d.ap()[:, 1024:2048])
            nc.scalar.dma_start(x_s[:, 2048:3072], x_d.ap()[:, 2048:3072])
            nc.scalar.dma_start(biasf[:, 2048:4096], b_d.ap()[:, 2048:4096])
            nc.scalar.dma_start(x_s[:, 3072:4096], x_d.ap()[:, 3072:4096])
            nc.gpsimd.dma_start(bv_s[:], bv_d.ap())

            # --- qkv projection: one K=128 M=48 matmul per 512-col
            # chunk, one DVE add folding all three biases, one SBUF->SBUF
            # DMA peeling k'' into the zero-padded stationary tile, one
            # xbar-transpose DMA building this chunk's v^T blocks.
            # Chunks 0-1 stage in acc_full[0:48] (freed early for the PV
            # accumulator); chunks 2-7 serialize through acc_full[64:112].
            def emit_proj(c):
                cs = slice(c * 512, (c + 1) * 512)
                if c % 2 == 0:
                    ps = acc_full[0:48, :]
                    tp = None
                else:
                    ps = acc_full[64:112, :]
                    tp = (0, 64)
                nc.tensor.matmul(ps, lhsT=w_s[:, 0:48], rhs=x_s[:, cs],
                                 start=True, stop=True, tile_position=tp)
                nc.vector.tensor_tensor(qz_all[0:3 * DH, cs], ps[:],
                                        biasf[:, cs], ADD)
                # k'' peel-off on the sync HWDGE queue, batched by
                # deadline (early chunks gate the exp stream start, late
                # ones only their own qk group); v^T transposes batched
                # per half ([16, 2048] -> [128, 16, 16]).
                if c in (0, 1, 3, 7):
                    lo = {0: 0, 1: 512, 3: 1024, 7: 2048}[c]
                    ks = slice(lo, (c + 1) * 512)
                    nc.sync.dma_start(kz_all[0:DH, ks],
                                      qz_all[DH:2 * DH, ks])
                if c in (3, 7):
                    ts2 = slice((c - 3) * 512, (c + 1) * 512)
                    nc.sync.dma_start_transpose(
                        va3[:, 4 * (c - 3):4 * (c + 1), 2 * DH:3 * DH],
                        qz_all[2 * DH:3 * DH, ts2])

            def make_pv(pt, jbs, acc, start, stop):
                def emit():
                    for t, jb in enumerate(jbs):
                        nc.tensor.matmul(
                            acc,
                            lhsT=vaugT[:, VS * jb:VS * (jb + 1)],
                            rhs=pt[:, 512 * t:512 * (t + 1)],
                            start=(start and t == 0),
                            stop=(stop and t == len(jbs) - 1),
                            skip_group_check=True)
                return emit

            def make_tail_a(acc):
                o17 = op.tile([3 * DH, ECOLS], f32, tag="o17")
                r = op.tile([1, ECOLS], f32, tag="r")

                def emit():
                    nc.vector.tensor_copy(o17[:], acc)
                    nc.vector.reciprocal_approx_fast(r[:], o17[0:1, :])
                return emit, o17, r

            def make_tail_b(o17, r, es):
                def emit():
                    # broadcast r across 16 partitions via a ones-matmul
                    # into spare partitions of the accumulator bank.
                    bc = acc_full[64:64 + DH, :]
                    nc.tensor.matmul(bc, lhsT=ones16[:], rhs=r[:],
                                     start=True, stop=True,
                                     tile_position=(0, 64),
                                     skip_group_check=True)
                    ost = op.tile([DH, ECOLS], f32, tag="ost")
                    nc.vector.tensor_tensor(ost[:], o17[2 * DH:3 * DH, :], bc, MULT)
                    nc.vector.tensor_scalar_add(ost[:], ost[:], bv_s[:])
                    nc.sync.dma_start(o_d.ap()[:, es], ost[:])
                return emit

            from collections import deque
            pend = deque()
            pending_a = None
            pending_b = None
            acc48 = acc_full[0:3 * DH, :]

            state = {"pending_a": None, "pending_b": None}

            def emit_group(e, gi, jbs, lag):
                njb = len(jbs)
                fw = 512 * njb
                if njb == 4:
                    st = stAp.tile([128, 2048], f32, tag="A")
                else:
                    st = stBp.tile([128, 1536], f32, tag="B")
                # Ready PV groups are emitted BEFORE this group's qk
                # matmuls: at eighth boundaries the A-tile WAR stalls the
                # first qk group ~1.3 us, and the in-order PE FIFO would
                # otherwise head-of-line-block the ready PV work behind
                # that stall.  Keep the most recent pending group back --
                # its exp may not have finished yet.
                while len(pend) >= max(lag, 2):
                    pend.popleft()()
                for t, jb in enumerate(jbs):
                    nc.tensor.matmul(
                        st[:, 512 * t:512 * (t + 1)],
                        lhsT=kz_all[:, jb * JW:(jb + 1) * JW],
                        rhs=qz_all[:, e * ECOLS:(e + 1) * ECOLS],
                        start=True, stop=True)
                pt = ptp.tile([128, fw], bf16, tag=("ptA" if njb == 4
                                                    else "ptB"))
                nc.scalar.activation(pt[:], st[:], EXP, bias=zerob[:])
                while len(pend) >= lag:
                    pend.popleft()()
                # The prior eighth's accumulator must be copied out
                # (tail_a) before this eighth's start=True PV clears it.
                # With lag==1 that clear is emitted at gi==1, so fire
                # tail_a at gi==0; with lag==3 it is emitted at gi==3 and
                # gi==2 keeps the copy off the PE's heels.
                tail_a_gi = 2 if lag >= 3 else 0
                if state["pending_a"] is not None and gi == tail_a_gi:
                    state["pending_a"]()
                    state["pending_a"] = None
                if state["pending_b"] is not None and gi == 6:
                    state["pending_b"]()
                    state["pending_b"] = None
                pend.append(make_pv(pt, jbs, acc48,
                                    start=(gi == 0),
                                    stop=(gi == len(GROUPS) - 1)))

            # Eighth 0 interleaves the tail of the projection chain with
            # the qk groups so the PE FIFO never sits behind the whole
            # chain; the first five chunks go up front (the alternating
            # even/odd staging regions let their extraction adds pipeline
            # at two chunks in flight).
            for c in range(5):
                emit_proj(c)
            for gi, jbs in enumerate(GROUPS):
                if gi < 3:
                    emit_proj(gi + 5)
                emit_group(0, gi, jbs, LAG)
            emit_a, o17, r = make_tail_a(acc48)
            state["pending_a"] = emit_a
            state["pending_b"] = make_tail_b(o17, r, slice(0, ECOLS))

            for e in range(1, NE):
                es = slice(e * ECOLS, (e + 1) * ECOLS)
                lag = LAG if e < NE - 1 else 1
                for gi, jbs in enumerate(GROUPS):
                    emit_group(e, gi, jbs, lag)
                while pend and e == NE - 1:
                    pend.popleft()()
                emit_a, o17, r = make_tail_a(acc48)
                if state["pending_a"] is not None:
                    state["pending_a"]()
                state["pending_a"] = emit_a
                if state["pending_b"] is not None:
                    state["pending_b"]()
                state["pending_b"] = make_tail_b(o17, r, es)
            while pend:
                pend.popleft()()
            state["pending_a"]()
            state["pending_b"]()

    nc.compile()
    return nc


def _get_program():
    global _compiled
    if _compiled is None:
        _compiled = _build_program()
    return _compiled


def _prepare_core_inputs(x, w_qkv, b_qkv, emb_d, emb_h, emb_w):
    x2 = np.ascontiguousarray(
        np.asarray(x, np.float32).reshape(C, N)).astype(np.float16)
    w_qkv = np.asarray(w_qkv, np.float32)
    b_qkv = np.asarray(b_qkv, np.float32)
    scale = DH ** -0.5
    emb = (np.asarray(emb_d, np.float32)
           + np.asarray(emb_h, np.float32)
           + np.asarray(emb_w, np.float32)).reshape(DH, N)
    in_maps = []
    for h in range(NHEADS):
        qc = slice(h * DH, (h + 1) * DH)
        kc = slice(DV + h * DH, DV + (h + 1) * DH)
        vc = slice(2 * DV + h * DH, 2 * DV + (h + 1) * DH)
        w = np.empty((C, 48), np.float32)
        w[:, 0:16] = w_qkv[:, qc] * scale
        w[:, 16:32] = w_qkv[:, kc]
        w[:, 32:48] = w_qkv[:, vc]
        w = w.astype(np.float16)
        bias = np.zeros((48, N), np.float32)
        bias[0:16, :] = (b_qkv[qc] * scale)[:, None]
        bias[16:32, :] = b_qkv[kc][:, None] + emb
        bias = bias.astype(np.float16)
        bv = np.ascontiguousarray(b_qkv[vc][:, None])
        in_maps.append({"x": x2, "w": w, "bias": bias, "bv": bv})
    return in_maps


def kernel(x, w_qkv, b_qkv, emb_d, emb_h, emb_w):
    from concourse.bass_utils import run_bass_kernel_spmd

    nc = _get_program()
    in_maps = _prepare_core_inputs(x, w_qkv, b_qkv, emb_d, emb_h, emb_w)
    res = run_bass_kernel_spmd(nc, in_maps, list(range(NHEADS)))
    out = np.empty((DV, N), np.float32)
    for h in range(NHEADS):
        out[h * DH:(h + 1) * DH, :] = res.results[h]["out"]
    return out.reshape(1, DV, 16, 16, 16)
